# revision 5
# baseline (speedup 1.0000x reference)
"""Self-contained Trainium2 Bass kernel for the SLAYER SNN problem (v2).

kernel(**inputs) takes FULL inputs {spikeInput:[64,4,2000], W1:[512,4],
W2:[2,512]} and returns the FULL [64,2,2000] output. Batch is sharded
8-ways across NeuronCores; each core runs an identical program on its
8 samples.

v2 restructure vs baseline:
- The layer-1 PSP filter is applied to the 4-channel *input* (2 full-T
  scans over [32,2000]) instead of the 512-channel fc1 output (1024
  per-block scans); CS is folded into W1 so the fc1 matmul directly
  produces the membrane drive P.
- The per-timestep refractory recurrence uses the scaled state
  g = yr/DR, collapsing 5 vector ops/step to 3:
      S  = (g*CR*DR >= TH - P)
      xr = DR*xr + S
      g  = DR*g + xr
- TH - P is produced on the Act engine straight from PSUM; the small
  layer-2/output scans run on the Pool engine. The DVE runs nothing but
  the 3-op step loop.
"""
from contextlib import ExitStack

import numpy as np

import concourse.bass as bass
import concourse.mybir as mybir
from concourse.bass_utils import run_bass_kernel_spmd
from concourse.tile import TileContext
import concourse.tile as _tile_mod
from concourse.vector_clock import ScopedClock as _ScopedClock, VectorClock as _VectorClock


def _drain_and_barrier_split(self, tick_clock, wait_clock):
    # Workaround for walrus "Too many sync wait commands" on the Tile tail
    # drain: emit one drain per processor instead of one multi-wait drain.
    gc = tick_clock.global_clock
    ticks = list(gc)
    for p, t in enumerate(ticks):
        if t <= 0:
            continue
        sub = [t if q == p else 0 for q in range(len(ticks))]
        drain_inst = self.nc.sync.drain()
        wait_clock.add_sem_waits(
            drain_inst.ins, _ScopedClock({None: _VectorClock(sub)}))
    self.nc.all_engine_barrier()
    assert self.sems is not None
    popped = self.nc._tile_sem_poison_stack.pop()
    assert popped is self._sem_poison
    self.nc.clear_and_free_semaphores(list(self.sems.allocated().values()))
    self.nc.all_engine_barrier()


_tile_mod.TileContext._drain_and_barrier = _drain_and_barrier_split


def _split_waits_json(raw):
    # walrus in this container accepts at most one sem-wait per instruction;
    # spill extras onto same-engine Drain carriers placed just before.
    import json as _json
    m = _json.loads(raw)
    ctr = 0
    for fn in m["functions"]:
        for bb in fn["blocks"]:
            out = []
            for i in bb.get("instructions", []):
                si = i.get("sync_info") or {}
                w = si.get("on_wait") or []
                if len(w) > 1:
                    for chunk in w[:-1]:
                        ctr += 1
                        out.append({
                            "debug": i.get("debug", 0), "engine": i["engine"],
                            "ins": [], "name": f"I-WS{ctr}", "opcode": "Drain",
                            "outs": [], "sync_info": {"on_wait": [chunk]},
                        })
                    si = dict(si)
                    si["on_wait"] = w[-1:]
                    i = dict(i)
                    i["sync_info"] = si
                out.append(i)
            bb["instructions"] = out
    return _json.dumps(m).encode()


def _install_wait_split(nc):
    orig = nc.to_json_bytes
    nc.to_json_bytes = lambda: _split_waits_json(orig())
    return nc


F32 = mybir.dt.float32
ALU = mybir.AluOpType
AF = mybir.ActivationFunctionType

DS = float(np.exp(np.float32(-1.0 / 10.0), dtype=np.float32))
DR = float(np.exp(np.float32(-1.0 / 1.0), dtype=np.float32))
CS = float(np.float32(np.e / 10.0))
CR = float(np.float32(-2.0 * 10.0 * np.e / 1.0))
CRDR = float(np.float32(CR) * np.float32(DR))
TH = 10.0
QINIT = 1e30  # disables the L2 column before its first real drive arrives

B = 64
N_CORES = 8
B_LOC = 8
NIN = 4
H = 512
HC = 4
NOUT = 2
NJ = 33  # 32 layer-1 columns + 1 layer-2 column
LAG = 2
T_FULL = 2000
L_BLK = 125

_nc_cache = {}


def build(T: int = T_FULL, L: int = L_BLK):
    NB = T // L
    assert NB * L == T and NB >= LAG
    HW = 4 * L  # 500: per-hc half-width for the fc2 matmul (<=512)
    nc = bass.Bass("TRN2", target_bir_lowering=False, debug=False,
                   num_devices=N_CORES)

    x_in = nc.declare_dram_parameter("x", [B_LOC * NIN, T], F32, isOutput=False)
    w1_in = nc.declare_dram_parameter("w1cs", [NIN, H], F32, isOutput=False)
    w2_in = nc.declare_dram_parameter("w2t", [128, HC * NOUT], F32, isOutput=False)
    out_d = nc.declare_dram_parameter("out", [B_LOC * NOUT, T], F32, isOutput=True)

    with TileContext(nc) as tc, ExitStack() as ctx:
        pool = ctx.enter_context(tc.tile_pool(name="main", bufs=1))
        psum = ctx.enter_context(tc.tile_pool(name="ps", bufs=1, space="PSUM"))

        w1 = pool.tile([NIN, H], F32, tag="w1", name="w1")
        nc.sync.dma_start(out=w1[:], in_=w1_in[:])
        w2t = pool.tile([128, HC * NOUT], F32, tag="w2t", name="w2t")
        nc.sync.dma_start(out=w2t[:], in_=w2_in[:])
        xraw = pool.tile([B_LOC * NIN, T], F32, tag="xraw", name="xraw")
        nc.sync.dma_start(out=xraw[:], in_=x_in[:])

        ds32 = pool.tile([B_LOC * NIN, T], F32, tag="ds32", name="ds32")
        nc.vector.memset(ds32[:], DS)
        thc = pool.tile([128, 1], F32, tag="thc", name="thc")
        nc.vector.memset(thc[:], TH)
        dsc16 = pool.tile([16, L], F32, tag="dsc16", name="dsc16")
        nc.vector.memset(dsc16[:], DS)

        # pre-update ys trace of the filtered input: ysx[:, t] = ys before
        # consuming x_t  (ysx[:, 0] = 0; scan writes post values at 1..T)
        xs32 = pool.tile([B_LOC * NIN, T], F32, tag="xs32", name="xs32")
        ysx = pool.tile([B_LOC * NIN, T + 1], F32, tag="ysx", name="ysx")
        nc.vector.memset(ysx[:], 0.0)
        nc.vector.tensor_tensor_scan(
            xs32[:], ds32[:], xraw[:], initial=0.0, op0=ALU.mult, op1=ALU.add)
        nc.vector.tensor_tensor_scan(
            ysx[:, 1:T + 1], xs32[:], ds32[:], initial=0.0,
            op0=ALU.add, op1=ALU.mult)
        # matmul rhs must start at partition 0: reshape [(b i), t] -> [i, (b t)]
        # via one gather-DMA per input channel (partition dim stays first).
        ysx2 = pool.tile([NIN, B_LOC * (T + 1)], F32, tag="ysx2", name="ysx2")
        for b in range(B_LOC):
            nc.sync.dma_start(
                out=ysx2[:, b * (T + 1):(b + 1) * (T + 1)],
                in_=ysx[b * NIN:(b + 1) * NIN, :])

        def zeros(shape, tag, eng=None):
            t = pool.tile(shape, F32, tag=tag, name=tag)
            (eng or nc.vector).memset(t[:], 0.0)
            return t

        Qb, Sb = [], []
        for i in range(2):
            q = pool.tile([128, NJ * L], F32, tag=f"Q{i}", name=f"Q{i}")
            nc.vector.memset(q[:], QINIT)
            Qb.append(q)
            Sb.append(zeros([128, NJ * L], f"S{i}"))

        xr_t = zeros([128, NJ], "xr")
        g_t = zeros([128, NJ], "g")

        a1_ps = [psum.tile([128, L], F32, tag=f"a1ps{i}", name=f"a1ps{i}")
                 for i in range(3)]
        a2_ps = [psum.tile([NOUT, 512], F32, tag=f"a2ps{i}", name=f"a2ps{i}")
                 for i in range(2)]
        a2_sb = [pool.tile([NOUT, 512], F32, tag=f"a2sb{i}", name=f"a2sb{i}")
                 for i in range(2)]
        a16 = [pool.tile([16, 63], F32, tag=f"a16{i}", name=f"a16{i}")
               for i in range(2)]
        x2 = zeros([16, 63], "x2")
        y2 = zeros([16, 64], "y2")
        x3 = zeros([16, L], "x3")
        y3 = zeros([16, L + 1], "y3")
        o3 = [pool.tile([16, L], F32, tag=f"o3{i}", name=f"o3{i}")
              for i in range(2)]

        def produce_L1(k):
            # fc1 drive for block k: P = W1cs @ ysx, Q = TH - P
            Q = Qb[k % 2]
            t0 = k * L
            for j in range(32):
                hc, b = divmod(j, B_LOC)
                ps = a1_ps[j % 3]
                nc.tensor.matmul(
                    ps[:],
                    lhsT=w1[:, hc * 128:(hc + 1) * 128],
                    rhs=ysx2[:, b * (T + 1) + t0:b * (T + 1) + t0 + L],
                    start=True, stop=True)
                nc.scalar.activation(
                    Q[:, j::NJ], ps[:], AF.Identity,
                    bias=thc[:], scale=-1.0)

        SUBW = (62, 63)

        def post_a2_sub(k, sub):
            # fc2 for the sub-half of block k -> a2_ps[sub] [2, (b,t)]
            t0 = 0 if sub == 0 else SUBW[0]
            w = SUBW[sub]
            ps = a2_ps[sub]
            S3 = Sb[k % 2][:].rearrange("p (t j) -> p j t", j=NJ)
            for hc in range(HC):
                nc.tensor.matmul(
                    ps[:, 0:B_LOC * w],
                    lhsT=w2t[:, hc * NOUT:(hc + 1) * NOUT],
                    rhs=S3[:, hc * B_LOC:(hc + 1) * B_LOC, t0:t0 + w],
                    start=(hc == 0), stop=(hc == HC - 1))

        def produce_sub(bb, sub):
            # Q2 for L2 times [bb*L + t0, +w) -> tile (bb+1)%2 positions
            # [t0, t0+w) of the L2 column (lag = one block).
            t0 = 0 if sub == 0 else SUBW[0]
            w = SUBW[sub]
            wprev = SUBW[1 - sub]
            ps = a2_ps[sub]
            sb2 = a2_sb[sub]
            a = a16[sub]
            Qt = Qb[(bb + 1) % 2]
            nc.scalar.copy(sb2[:, 0:B_LOC * w], ps[:, 0:B_LOC * w])
            for b in range(B_LOC):
                nc.sync.dma_start(out=a[2 * b:2 * b + 2, 0:w],
                                  in_=sb2[:, b * w:(b + 1) * w])
            nc.vector.tensor_tensor_scan(
                x2[:, 0:w], dsc16[:, 0:w], a[:, 0:w],
                initial=x2[:, wprev - 1:wprev], op0=ALU.mult, op1=ALU.add)
            nc.scalar.activation(
                Qt[0:16, t0 * NJ + 32:t0 * NJ + 33], y2[:, wprev:wprev + 1],
                AF.Identity, bias=thc[0:16], scale=-CS)
            nc.vector.tensor_tensor_scan(
                y2[:, 1:w + 1], x2[:, 0:w], dsc16[:, 0:w],
                initial=y2[:, wprev:wprev + 1], op0=ALU.add, op1=ALU.mult)
            nc.scalar.activation(
                Qt[0:16, (t0 + 1) * NJ + 32:(t0 + w - 1) * NJ + 33:NJ],
                y2[:, 1:w], AF.Identity, bias=thc[0:16], scale=-CS)

        def step(k, tau, narrow=False):
            Q, S = Qb[k % 2], Sb[k % 2]
            if narrow:
                Qc = Q[0:16, tau * NJ + 32:tau * NJ + 33]
                Sc = S[0:16, tau * NJ + 32:tau * NJ + 33]
                g, x = g_t[0:16, 32:33], xr_t[0:16, 32:33]
            else:
                Qc = Q[:, tau * NJ:(tau + 1) * NJ]
                Sc = S[:, tau * NJ:(tau + 1) * NJ]
                g, x = g_t[:], xr_t[:]
            nc.vector.scalar_tensor_tensor(Sc, g, CRDR, Qc, ALU.mult, ALU.is_ge)
            nc.vector.scalar_tensor_tensor(x, x, DR, Sc, ALU.mult, ALU.add)
            nc.vector.scalar_tensor_tensor(g, g, DR, x, ALU.mult, ALU.add)

        def post_out(k):
            # final psp + store for L2 time block k-LAG
            S = Sb[k % 2]
            o = o3[k % 2]
            s2 = S[0:16, 32::NJ]
            nc.vector.tensor_tensor_scan(
                x3[:], dsc16[:], s2,
                initial=x3[:, L - 1:L], op0=ALU.mult, op1=ALU.add)
            nc.scalar.mul(o[:, 0:1], y3[:, L:L + 1], CS)
            nc.vector.tensor_tensor_scan(
                y3[:, 1:L + 1], x3[:], dsc16[:],
                initial=y3[:, L:L + 1], op0=ALU.add, op1=ALU.mult)
            nc.scalar.mul(o[:, 1:L], y3[:, 1:L], CS)
            nc.sync.dma_start(out=out_d[:, (k - 1) * L:k * L],
                              in_=o[:])

        produce_L1(0)
        for k in range(NB + 1):
            wide = k < NB
            if k + 1 < NB:
                produce_L1(k + 1)
            for tau in range(0, SUBW[0]):
                if tau == 8 and k >= 1:
                    produce_sub(k - 1, 1)
                step(k, tau, narrow=not wide)
            if wide:
                post_a2_sub(k, 0)
            for tau in range(SUBW[0], L):
                if tau == 70 and wide:
                    produce_sub(k, 0)
                step(k, tau, narrow=not wide)
            if wide:
                post_a2_sub(k, 1)
            if k >= 1:
                post_out(k)

    return _install_wait_split(nc)


def host_prep(spikeInput, W1, W2, core):
    b0 = core * B_LOC
    x = np.ascontiguousarray(
        spikeInput[b0:b0 + B_LOC].reshape(B_LOC * NIN, T_FULL)).astype(np.float32)
    w1cs = np.ascontiguousarray(
        (np.float32(CS) * W1.astype(np.float32)).T).astype(np.float32)
    w2t = np.empty((128, HC * NOUT), np.float32)
    for hcc in range(HC):
        for o in range(NOUT):
            w2t[:, hcc * NOUT + o] = W2[o, hcc * 128:(hcc + 1) * 128]
    return {"x": x, "w1cs": w1cs, "w2t": w2t}


def _get_nc():
    if "nc" not in _nc_cache:
        _nc_cache["nc"] = build()
    return _nc_cache["nc"]


def kernel(spikeInput=None, W1=None, W2=None, _trace=False, **kw):
    spikeInput = np.asarray(spikeInput, dtype=np.float32)
    W1 = np.asarray(W1, dtype=np.float32)
    W2 = np.asarray(W2, dtype=np.float32)
    nc = _get_nc()
    in_maps = [host_prep(spikeInput, W1, W2, c) for c in range(N_CORES)]
    res = run_bass_kernel_spmd(nc, in_maps, list(range(N_CORES)), trace=_trace)
    out = np.empty((B, NOUT, T_FULL), np.float32)
    for c in range(N_CORES):
        o = res.results[c]["out"].reshape(B_LOC, NOUT, T_FULL)
        out[c * B_LOC:(c + 1) * B_LOC] = o
    if _trace:
        return out, res
    return out
